# revision 1
# baseline (speedup 1.0000x reference)
"""Trainium2 Bass kernel for nn_Explore_Decoder (scatter_memory).

Full computation:
    a      = all_memory @ U_w                         [B,S,H]
    l      = (last_memory @ W_w)[:,None,:]            [B,1,H]
    scores = (tanh(a+l) @ V_w + V_b)[...,0]           [B,S]
    scores = where(mask, -1e9, scores)
    alpha  = softmax(scores, axis=1)
    out_e  = sum(alpha * all_memory, axis=1)          [B,H]
    feats  = concat([out_e, last_memory], axis=1)     [B,2H]
    logits = feats @ E_w                              [B,N]
    logits = where(seen_item, -inf, logits)           (scatter of item_seq)
    return sigmoid(logits)

Sharding (8 cores):
  Phase 1 (attention): data-parallel over B; core c owns rows [c*128,(c+1)*128).
    Produces featsT [2H, 128] per core, AllGather -> featsT for all B on every core.
  Phase 2 (logits): tensor-parallel over N; core c owns cols [c*6250,(c+1)*6250).
    Dense: out = sigmoid(featsT.T @ E_w[:, cols_c]) for all 1024 rows.
    Scatter: host-precomputed flat offsets of seen items in the local column
    range; indirect DMA writes 0.0 ( = sigmoid(-inf)) over the dense output.
"""

import numpy as np

B, S, H, N = 1024, 100, 128, 50000
NCORES = 8
BL = B // NCORES          # 128 batch rows per core (phase 1)
NL = N // NCORES          # 6250 vocab cols per core (phase 2)
H2 = 2 * H
SB = 4                    # s-values per phase-1 block
NSB = S // SB             # 25 blocks
SENTINEL = np.int32(2**30)

_BUILT = {}               # scat_w -> compiled Bass module
_LAST_RESULTS = None      # BassKernelResults of the most recent run (for tests)


def _default_spec(n_scat=100):
    # representative scatter spec for profiling: uniform item distribution
    per = 12800 / NCORES
    dep = tuple(min(NCORES - 1, int((w + 1) * 128 / per)) for w in range(n_scat))
    return (n_scat, dep)


def _build(scat_w, reps: int = 1, timeline: bool = False,
           no_scatter: bool = False, two_mm: bool = False,
           use_f32r: bool = False):
    # scat_w: (n_insts, dep_cb tuple) from host packing
    import concourse.bass as bass
    import concourse.mybir as mybir
    import concourse.tile as tile
    from concourse import bacc
    from concourse.masks import make_identity

    f32 = mybir.dt.float32
    bf16 = mybir.dt.bfloat16
    i32 = mybir.dt.int32
    AF = mybir.ActivationFunctionType
    ALU = mybir.AluOpType
    AX = mybir.AxisListType

    nc = bacc.Bacc(None, target_bir_lowering=False, debug=False)

    am = nc.dram_tensor("am", [BL, S * H], f32, kind="ExternalInput")
    lm = nc.dram_tensor("lm", [BL, H], f32, kind="ExternalInput")
    maskb = nc.dram_tensor("maskb", [BL, S], f32, kind="ExternalInput")
    uw = nc.dram_tensor("uw", [H, H], bf16, kind="ExternalInput")
    ww = nc.dram_tensor("ww", [H, H], bf16, kind="ExternalInput")
    vw = nc.dram_tensor("vw", [H, 1], bf16, kind="ExternalInput")
    ew = nc.dram_tensor("ew", [H2, NL], bf16, kind="ExternalInput")
    ewl = nc.dram_tensor("ewl", [H2, NL], bf16, kind="ExternalInput")
    f32r = mybir.dt.float32r
    ewf = (nc.dram_tensor("ewf", [H2, NL], f32r, kind="ExternalInput")
           if use_f32r else None)
    n_scat, dep_cb = scat_w
    # [p, w] = offset for partition p of indirect-DMA instruction w;
    # instruction w may fire once the store of block dep_cb[w] has landed
    sidx = nc.dram_tensor("sidx", [128, n_scat], i32, kind="ExternalInput")
    out = nc.dram_tensor("out", [B, NL], f32, kind="ExternalOutput")
    # timeline mode: single-core cost-model sim can't price collectives or
    # full-tensor indirect APs; swap in traffic-equivalent stand-ins
    dumout = nc.dram_tensor("dumout", [128, 1], f32) if timeline else None

    with tile.TileContext(nc) as tc:
      for _rep in range(reps):
        with (
            tc.tile_pool(name="consts", bufs=1) as cp,
            tc.tile_pool(name="amp", bufs=1) as amp,
            tc.tile_pool(name="ewp", bufs=1) as ewp,
            tc.tile_pool(name="dram", bufs=1, space="DRAM") as dp,
            tc.tile_pool(name="smax", bufs=1) as sm,
        ):
            ident = cp.tile([128, 128], f32)
            make_identity(nc, ident[:])
            uw_sb = cp.tile([H, H], bf16)
            nc.sync.dma_start(out=uw_sb[:], in_=uw[:, :])
            ww_sb = cp.tile([H, H], bf16)
            nc.sync.dma_start(out=ww_sb[:], in_=ww[:, :])
            vw_sb = cp.tile([H, 1], bf16)
            nc.sync.dma_start(out=vw_sb[:], in_=vw[:, :])
            maskb_sb = cp.tile([BL, S], f32)
            nc.sync.dma_start(out=maskb_sb[:], in_=maskb[:, :])
            lm_sb = cp.tile([BL, H], f32)
            nc.sync.dma_start(out=lm_sb[:], in_=lm[:, :])

            am_t = amp.tile([BL, S * H], f32)
            AMCH = 20 * H
            for a0 in range(0, S * H, AMCH):
                nc.sync.dma_start(out=am_t[:, a0:a0 + AMCH],
                                  in_=am[:, a0:a0 + AMCH])
            amv = am_t[:].rearrange("p (s h) -> p s h", h=H)

            feats_local = dp.tile([H2, BL], f32)
            gath = dp.tile([NCORES * H2, BL], f32)

            # ---------------- Phase 1: attention over S, rows of this core ----
            with (
                tc.tile_pool(name="ps_t", bufs=3, space="PSUM") as ps_t,
                tc.tile_pool(name="ps_z", bufs=2, space="PSUM") as ps_z,
                tc.tile_pool(name="ps_acc", bufs=1, space="PSUM") as ps_acc,
                tc.tile_pool(name="xtp", bufs=6) as xtp,
                tc.tile_pool(name="tzp", bufs=6) as tzp,
            ):
                # last_memory^T  [H, BL] , replicated x SB for the Z matmul rhs
                lmT_ps = ps_t.tile([128, 512], f32, tag="tps")
                nc.tensor.transpose(out=lmT_ps[:, :H], in_=lm_sb[:],
                                    identity=ident[:])
                lmT_sb = cp.tile([H, BL], f32)
                nc.vector.tensor_copy(lmT_sb[:], lmT_ps[:, :H])
                lmT_rep = cp.tile([H, SB * BL], bf16)
                nc.vector.tensor_copy(
                    lmT_rep[:].rearrange("h (s b) -> h s b", s=SB),
                    lmT_sb[:].unsqueeze(1).broadcast_to([H, SB, BL]),
                )
                # feats rows H..2H = last_memory^T (raw)
                nc.sync.dma_start(out=feats_local[H:H2, :], in_=lmT_sb[:])

                sc_ps = ps_acc.tile([BL, S], f32, tag="sc")
                for sb in range(NSB):
                    xt_ps = ps_t.tile([128, SB * 128], f32, tag="tps")
                    for j in range(SB):
                        s = sb * SB + j
                        nc.tensor.transpose(
                            out=xt_ps[:, j * 128:(j + 1) * 128],
                            in_=amv[:, s, :],
                            identity=ident[:],
                        )
                    xt = xtp.tile([128, SB * 128], bf16)
                    if sb % 2 == 0:
                        nc.vector.tensor_copy(xt[:], xt_ps[:])
                    else:
                        nc.scalar.copy(xt[:], xt_ps[:])
                    z_ps = ps_z.tile([128, SB * BL], f32)
                    nc.tensor.matmul(z_ps[:], lhsT=uw_sb[:], rhs=xt[:],
                                     start=True, stop=False)
                    nc.tensor.matmul(z_ps[:], lhsT=ww_sb[:], rhs=lmT_rep[:],
                                     start=False, stop=True)
                    tz = tzp.tile([128, SB * BL], bf16)
                    nc.scalar.activation(tz[:], z_ps[:], AF.Tanh)
                    for j in range(SB):
                        s = sb * SB + j
                        nc.tensor.matmul(
                            sc_ps[:, s:s + 1],
                            lhsT=tz[:, j * 128:(j + 1) * 128],
                            rhs=vw_sb[:, 0:1],
                            start=True, stop=True,
                        )

                # softmax over S (per row), normalization folded into alpha
                sc_sb = sm.tile([BL, S], f32)
                nc.vector.tensor_tensor(sc_sb[:], sc_ps[:], maskb_sb[:], op=ALU.add)
                neg_mx = sm.tile([BL, 1], f32)
                nc.vector.reduce_max(neg_mx[:], sc_sb[:], AX.X, negate=True)
                expsc = sm.tile([BL, S], f32)
                sum_sb = sm.tile([BL, 1], f32)
                nc.scalar.activation(expsc[:], sc_sb[:], AF.Exp,
                                     bias=neg_mx[:, 0:1], accum_out=sum_sb[:, 0:1])
                rsum = sm.tile([BL, 1], f32)
                nc.vector.reciprocal(rsum[:], sum_sb[:])
                alpha = sm.tile([BL, S], f32)
                nc.vector.tensor_scalar_mul(alpha[:], expsc[:], rsum[:, 0:1])

                # weighted memory: am *= alpha (broadcast over H), chunked over S
                CH = 20
                for c0 in range(0, S, CH):
                    nc.vector.tensor_tensor(
                        amv[:, c0:c0 + CH, :],
                        amv[:, c0:c0 + CH, :],
                        alpha[:, c0:c0 + CH].unsqueeze(2).broadcast_to([BL, CH, H]),
                        op=ALU.mult,
                    )
                # out_e^T [H, BL] = sum_s (alpha*am)_s^T  via PE transposes into PSUM
                oe_ps = ps_acc.tile([H, BL], f32, tag="oe")
                for s in range(S):
                    nc.tensor.matmul(oe_ps[:], lhsT=amv[:, s, :], rhs=ident[:],
                                     start=(s == 0), stop=(s == S - 1),
                                     is_transpose=True)
                oeT_sb = sm.tile([H, BL], f32)
                nc.vector.tensor_copy(oeT_sb[:], oe_ps[:])
                nc.sync.dma_start(out=feats_local[0:H, :], in_=oeT_sb[:])

            # ---------------- AllGather feats ----------------
            if timeline:
                gvw = gath[:].rearrange("(c f) b -> c f b", f=H2)
                for c in range(NCORES):
                    nc.sync.dma_start(out=gvw[c], in_=feats_local[:])
            else:
                nc.gpsimd.collective_compute(
                    "AllGather",
                    mybir.AluOpType.bypass,
                    replica_groups=[list(range(NCORES))],
                    ins=[feats_local[:].opt()],
                    outs=[gath[:].opt()],
                )
            gv = gath[:].rearrange("(c f) b -> c f b", f=H2)

            # ---------------- Phase 2: logits + sigmoid over local cols -------
            if not use_f32r:
                ew_top = ewp.tile([H, NL], bf16)
                nc.sync.dma_start(out=ew_top[:], in_=ew[0:H, :])
                ew_bot = ewp.tile([H, NL], bf16)
                nc.sync.dma_start(out=ew_bot[:], in_=ew[H:H2, :])
                ewl_top = ewp.tile([H, NL], bf16)
                nc.sync.dma_start(out=ewl_top[:], in_=ewl[0:H, :])
                ewl_bot = ewp.tile([H, NL], bf16)
                nc.sync.dma_start(out=ewl_bot[:], in_=ewl[H:H2, :])
            if use_f32r:
                ewr_top = ewp.tile([H, NL], f32r)
                nc.sync.dma_start(out=ewr_top[:], in_=ewf[0:H, :])
                ewr_bot = ewp.tile([H, NL], f32r)
                nc.sync.dma_start(out=ewr_bot[:], in_=ewf[H:H2, :])

            if timeline:
                out_flat = dumout[:, :]
            else:
                out_flat = out[:, :].rearrange("a b -> (a b)").unsqueeze(1)

            def scat_ap(k):
                # unique fake dep region per scatter: suppresses Tile's
                # WAW serialization between scatters (all write 0.0 at
                # host-guaranteed positions; order among them is free).
                # The real store->scatter ordering is added explicitly.
                return bass.AP(
                    tensor=out_flat.tensor, offset=0, ap=out_flat.ap,
                    dep_tracking_offset=(1 << 33) + k * (1 << 23))

            zeros_sb = None

            with (
                tc.tile_pool(name="ps2", bufs=8, space="PSUM") as ps2,
                tc.tile_pool(name="outp", bufs=2) as outp,
                tc.tile_pool(name="gp", bufs=2) as gp,
            ):
                zeros_sb = cp.tile([128, 1], f32)
                nc.vector.memset(zeros_sb[:], 0.0)
                NCHW = 512
                g_all_f = gp.tile([128, 2 * NCORES * 128], f32)
                nc.sync.dma_start(
                    out=g_all_f[:],
                    in_=gath[:].rearrange("(t p) b -> p t b", p=128))
                if not use_f32r:
                    g_all = gp.tile([128, 2 * NCORES * 128], bf16)
                    nc.vector.tensor_copy(g_all[:], g_all_f[:])
                if use_f32r:
                    g_r = gp.tile([128, 2 * NCORES * 128], f32r)
                    nc.sync.dma_start(
                        out=g_r[:],
                        in_=gath[:].rearrange("(t p) b -> p t b",
                                              p=128).bitcast(f32r))
                if not use_f32r:
                    g_rs = gp.tile([128, 2 * NCORES * 128], f32)
                    nc.vector.tensor_tensor(g_rs[:], g_all_f[:], g_all[:],
                                            op=ALU.subtract)
                    g_lo = gp.tile([128, 2 * NCORES * 128], bf16)
                    nc.vector.tensor_copy(g_lo[:], g_rs[:])
                six_all = gp.tile([128, n_scat], i32)
                nc.sync.dma_start(out=six_all[:], in_=sidx[:, :])
                scat_by_dep = {}
                for w, d in enumerate(dep_cb):
                    scat_by_dep.setdefault(d, []).append(w)
                for cb in range(NCORES):
                    if not use_f32r:
                        g_oe = g_all[:, (2 * cb) * 128:(2 * cb + 1) * 128]
                        g_lm = g_all[:, (2 * cb + 1) * 128:(2 * cb + 2) * 128]
                        gl_oe = g_lo[:, (2 * cb) * 128:(2 * cb + 1) * 128]
                        gl_lm = g_lo[:, (2 * cb + 1) * 128:(2 * cb + 2) * 128]
                    out_sb = outp.tile([128, NL], f32)
                    for n0 in range(0, NL, NCHW):
                        w = min(NCHW, NL - n0)
                        pt = ps2.tile([128, NCHW], f32)
                        for q0 in range(0, w, 512):
                            qw = min(512, w - q0)
                            sl = slice(n0 + q0, n0 + q0 + qw)
                            po = pt[:, q0:q0 + qw]
                            if use_f32r:
                                gr_oe = g_r[:, (2 * cb) * 128:(2 * cb + 1) * 128]
                                gr_lm = g_r[:, (2 * cb + 1) * 128:(2 * cb + 2) * 128]
                                nc.tensor.matmul(po, lhsT=gr_oe,
                                                 rhs=ewr_top[:, sl],
                                                 start=True, stop=False)
                                nc.tensor.matmul(po, lhsT=gr_lm,
                                                 rhs=ewr_bot[:, sl],
                                                 start=False, stop=True)
                                continue
                            if two_mm:
                                nc.tensor.matmul(po, lhsT=g_oe,
                                                 rhs=ew_top[:, sl],
                                                 start=True, stop=False)
                                nc.tensor.matmul(po, lhsT=g_lm,
                                                 rhs=ew_bot[:, sl],
                                                 start=False, stop=True)
                                continue
                            nc.tensor.matmul(po, lhsT=g_oe,
                                             rhs=ew_top[:, sl],
                                             start=True, stop=False)
                            nc.tensor.matmul(po, lhsT=g_oe,
                                             rhs=ewl_top[:, sl],
                                             start=False, stop=False)
                            nc.tensor.matmul(po, lhsT=gl_oe,
                                             rhs=ew_top[:, sl],
                                             start=False, stop=False)
                            nc.tensor.matmul(po, lhsT=g_lm,
                                             rhs=ew_bot[:, sl],
                                             start=False, stop=False)
                            nc.tensor.matmul(po, lhsT=g_lm,
                                             rhs=ewl_bot[:, sl],
                                             start=False, stop=False)
                            nc.tensor.matmul(po, lhsT=gl_lm,
                                             rhs=ew_bot[:, sl],
                                             start=False, stop=True)
                        nc.scalar.activation(out_sb[:, n0:n0 + w], pt[:, :w],
                                             AF.Sigmoid)
                    st = nc.sync.dma_start(out=out[cb * 128:(cb + 1) * 128, :],
                                           in_=out_sb[:])
                    # seen-item mask: scatter 0.0 over stored rows. HW indirect
                    # DMA consumes one offset per partition -> 128 single-
                    # element writes per instruction; instruction w carries
                    # offsets only from row-blocks <= dep_cb[w].
                    for w in ([] if no_scatter else scat_by_dep.get(cb, [])):
                        sc_inst = nc.gpsimd.indirect_dma_start(
                            out=scat_ap(w),
                            out_offset=bass.IndirectOffsetOnAxis(
                                ap=six_all[:, w:w + 1], axis=0),
                            in_=zeros_sb[:, :],
                            in_offset=None,
                            bounds_check=(127 if timeline else B * NL - 1),
                            oob_is_err=False,
                        )
                        tile.add_dep_helper(sc_inst.ins, st.ins,
                                            reason="scatter after dense store")

    nc.compile()
    return nc


def _prepare_inputs(all_memory, last_memory, item_seq, mask, U_w, W_w, V_w, E_w):
    all_memory = np.asarray(all_memory, dtype=np.float32)
    last_memory = np.asarray(last_memory, dtype=np.float32)
    item_seq = np.asarray(item_seq)
    mask = np.asarray(mask)
    import ml_dtypes
    U_w = np.ascontiguousarray(np.asarray(U_w, dtype=np.float32).astype(ml_dtypes.bfloat16))
    W_w = np.ascontiguousarray(np.asarray(W_w, dtype=np.float32).astype(ml_dtypes.bfloat16))
    V_w = np.ascontiguousarray(np.asarray(V_w, dtype=np.float32).reshape(H, 1).astype(ml_dtypes.bfloat16))
    E_w32 = np.asarray(E_w, dtype=np.float32)
    E_w = E_w32.astype(ml_dtypes.bfloat16)
    E_wlo = (E_w32 - E_w.astype(np.float32)).astype(ml_dtypes.bfloat16)

    # ----- host-side scatter index prep (per core, per 128-row block) -----
    items = item_seq.astype(np.int64)
    valid = items > 0
    core_of = items // NL
    b_idx = np.arange(B)[:, None].repeat(S, axis=1)
    flat_in_core = b_idx * NL + (items - core_of * NL)   # [B,S]

    # pack each core's offsets cb-ordered into chunks of 128 (one indirect-DMA
    # instruction each); record which store each chunk must wait for
    offs = {}
    for c in range(NCORES):
        for cb in range(NCORES):
            sel = valid & (core_of == c) & ((b_idx // 128) == cb)
            offs[(c, cb)] = flat_in_core[sel].astype(np.int32)
    totals = [sum(offs[(c, cb)].size for cb in range(NCORES))
              for c in range(NCORES)]
    n_scat = max(2, -(-max(totals) // 128))
    sidx_all = np.full((NCORES, 128, n_scat), SENTINEL, dtype=np.int32)
    dep = np.zeros((NCORES, n_scat), dtype=np.int64)
    for c in range(NCORES):
        flat = np.full(n_scat * 128, SENTINEL, dtype=np.int32)
        cbs = np.zeros(n_scat * 128, dtype=np.int64)
        pos = 0
        for cb in range(NCORES):
            o = offs[(c, cb)]
            flat[pos:pos + o.size] = o
            cbs[pos:pos + o.size] = cb
            pos += o.size
        sidx_all[c] = flat.reshape(n_scat, 128).T
        dep[c] = cbs.reshape(n_scat, 128).max(axis=1)
    dep_cb = tuple(int(x) for x in dep.max(axis=0))
    scat_w = (n_scat, dep_cb)

    maskbias = np.where(mask, np.float32(-1e9), np.float32(0.0)).astype(np.float32)
    in_maps = []
    for c in range(NCORES):
        r0, r1 = c * BL, (c + 1) * BL
        in_maps.append({
            "am": np.ascontiguousarray(
                all_memory[r0:r1].reshape(BL, S * H)),
            "lm": np.ascontiguousarray(last_memory[r0:r1]),
            "maskb": np.ascontiguousarray(maskbias[r0:r1]),
            "uw": U_w,
            "ww": W_w,
            "vw": V_w,
            "ew": np.ascontiguousarray(E_w[:, c * NL:(c + 1) * NL]),
            "ewl": np.ascontiguousarray(E_wlo[:, c * NL:(c + 1) * NL]),
            "sidx": np.ascontiguousarray(sidx_all[c]),
        })
    return scat_w, in_maps


def kernel(all_memory, last_memory, item_seq, mask, U_w, W_w, V_w, V_b, E_w):
    from concourse.bass_utils import run_bass_kernel_spmd

    scat_w, in_maps = _prepare_inputs(
        all_memory, last_memory, item_seq, mask, U_w, W_w, V_w, E_w)
    if scat_w not in _BUILT:
        _BUILT[scat_w] = _build(scat_w)
    nc = _BUILT[scat_w]
    res = run_bass_kernel_spmd(nc, in_maps, core_ids=list(range(NCORES)))
    global _LAST_RESULTS
    _LAST_RESULTS = res
    return np.concatenate([res.results[c]["out"] for c in range(NCORES)], axis=1)



# revision 5
# speedup vs baseline: 1.5218x; 1.5218x over previous
"""Trainium2 Bass kernel for nn_Explore_Decoder (scatter_memory).

Full computation:
    a      = all_memory @ U_w                         [B,S,H]
    l      = (last_memory @ W_w)[:,None,:]            [B,1,H]
    scores = (tanh(a+l) @ V_w + V_b)[...,0]           [B,S]
    scores = where(mask, -1e9, scores)
    alpha  = softmax(scores, axis=1)
    out_e  = sum(alpha * all_memory, axis=1)          [B,H]
    feats  = concat([out_e, last_memory], axis=1)     [B,2H]
    logits = feats @ E_w                              [B,N]
    logits = where(seen_item, -inf, logits)           (scatter of item_seq)
    return sigmoid(logits)

Sharding (8 cores):
  Phase 1 (attention): data-parallel over B; core c owns rows [c*128,(c+1)*128).
    Produces featsT [2H, 128] bf16 per core, AllGather -> featsT for all B.
  Phase 2 (logits): tensor-parallel over N; core c owns cols [c*6250,(c+1)*6250).
    Dense: out = sigmoid(featsT.T @ E_w[:, cols_c]) for all 1024 rows, bf16
    matmuls (rel-err gate is 2e-2; bf16 noise is ~1e-3), output stored bf16
    and upcast to f32 on the host.
    Seen-item mask: gpsimd local_scatter builds a per-row-block hit mask
    (-1 at host-precomputed local columns, 0 elsewhere) on the otherwise-idle
    Pool engine, concurrent with phase 1; a fused DVE op applies
    out = (mask + 1) * sigmoid  (so seen positions become exactly 0.0).
"""

import numpy as np

B, S, H, N = 1024, 100, 128, 50000
NCORES = 8
BL = B // NCORES          # 128 batch rows per core (phase 1)
NL = N // NCORES          # 6250 vocab cols per core (phase 2)
H2 = 2 * H
SB = 4                    # s-values per phase-1 block
NSB = S // SB             # 25 blocks
# local_scatter windows covering the NL=6250 local columns
# (num_elems*32 < 2^16 caps a window at 2046)
WINDOWS = [(0, 2046), (2046, 2046), (4092, 2046), (6138, 112)]
NW = len(WINDOWS)

_BUILT = {}               # scat_w -> compiled Bass module
_LAST_RESULTS = None      # BassKernelResults of the most recent run (for tests)


def _default_spec(NI=16):
    # NI = index slots per (row, window) for the mask local_scatters
    return (NI,)


def _build(scat_w, reps: int = 1, timeline: bool = False):
    # scat_w: (NI,) index slots per (row, window)
    import concourse.bass as bass
    import concourse.mybir as mybir
    import concourse.tile as tile
    from concourse import bacc
    from concourse.masks import make_identity

    f32 = mybir.dt.float32
    bf16 = mybir.dt.bfloat16
    i16 = mybir.dt.int16
    AF = mybir.ActivationFunctionType
    ALU = mybir.AluOpType
    AX = mybir.AxisListType

    nc = bacc.Bacc(None, target_bir_lowering=False, debug=False)

    am = nc.dram_tensor("am", [BL, S * H], f32, kind="ExternalInput")
    lm = nc.dram_tensor("lm", [BL, H], f32, kind="ExternalInput")
    maskb = nc.dram_tensor("maskb", [BL, S], f32, kind="ExternalInput")
    uw = nc.dram_tensor("uw", [H, H], bf16, kind="ExternalInput")
    ww = nc.dram_tensor("ww", [H, H], bf16, kind="ExternalInput")
    vw = nc.dram_tensor("vw", [H, 1], bf16, kind="ExternalInput")
    ew = nc.dram_tensor("ew", [H2, NL], bf16, kind="ExternalInput")
    (NI,) = scat_w
    # [p, ((cb*NW + w)*NI + k)] = local column (int16) of the k-th seen item
    # of row cb*128+p inside window w, or -1 (ignored)
    sidx = nc.dram_tensor("sidx", [128, NCORES * NW * NI], i16,
                          kind="ExternalInput")
    out = nc.dram_tensor("out", [B, NL], bf16, kind="ExternalOutput")

    with tile.TileContext(nc) as tc:
      for _rep in range(reps):
        with (
            tc.tile_pool(name="consts", bufs=1) as cp,
            tc.tile_pool(name="ewp", bufs=1) as ewp,
            tc.tile_pool(name="dram", bufs=1, space="DRAM") as dp,
            tc.tile_pool(name="smax", bufs=1) as sm,
            tc.tile_pool(name="mkp", bufs=6) as mkp,
        ):
            ident = cp.tile([128, 128], f32)
            make_identity(nc, ident[:])
            uw_sb = cp.tile([H, H], bf16)
            nc.sync.dma_start(out=uw_sb[:], in_=uw[:, :])
            ww_sb = cp.tile([H, H], bf16)
            nc.sync.dma_start(out=ww_sb[:], in_=ww[:, :])
            vw_sb = cp.tile([H, 1], bf16)
            nc.sync.dma_start(out=vw_sb[:], in_=vw[:, :])
            maskb_sb = cp.tile([BL, S], f32)
            nc.sync.dma_start(out=maskb_sb[:], in_=maskb[:, :])
            lm_sb = cp.tile([BL, H], f32)
            nc.sync.dma_start(out=lm_sb[:], in_=lm[:, :])

            # seen-item masks: built on Pool, consumed in phase 2
            six_all = cp.tile([128, NCORES * NW * NI], i16)
            nc.sync.dma_start(out=six_all[:], in_=sidx[:, :])
            neg1 = cp.tile([128, NI], bf16)
            nc.vector.memset(neg1[:], -1.0)
            masks = []
            for cb in range(NCORES):
                mk = mkp.tile([128, NL], bf16, name="mk", tag="mk")
                for w, (w0, wlen) in enumerate(WINDOWS):
                    i0 = (cb * NW + w) * NI
                    nc.gpsimd.local_scatter(
                        mk[:, w0:w0 + wlen],
                        neg1[:],
                        six_all[:, i0:i0 + NI],
                        channels=128, num_elems=wlen, num_idxs=NI,
                    )
                masks.append(mk)

            feats_local = dp.tile([H2, BL], bf16)
            gath = dp.tile([NCORES * H2, BL], bf16)

            # ---------------- Phase 1: attention over S, rows of this core ----
            with (
                tc.tile_pool(name="amp", bufs=1) as amp,
                tc.tile_pool(name="ps_t", bufs=3, space="PSUM") as ps_t,
                tc.tile_pool(name="ps_z", bufs=2, space="PSUM") as ps_z,
                tc.tile_pool(name="ps_acc", bufs=1, space="PSUM") as ps_acc,
                tc.tile_pool(name="xtp", bufs=6) as xtp,
                tc.tile_pool(name="tzp", bufs=6) as tzp,
            ):
                am_t = amp.tile([BL, S * H], f32)
                AMCH = 20 * H
                for a0 in range(0, S * H, AMCH):
                    nc.sync.dma_start(out=am_t[:, a0:a0 + AMCH],
                                      in_=am[:, a0:a0 + AMCH])
                amv = am_t[:].rearrange("p (s h) -> p s h", h=H)

                # last_memory^T  [H, BL] , replicated x SB for the Z matmul rhs
                lmT_ps = ps_t.tile([128, 512], f32, tag="tps")
                nc.tensor.transpose(out=lmT_ps[:, :H], in_=lm_sb[:],
                                    identity=ident[:])
                lmT_sb = cp.tile([H, BL], f32)
                nc.vector.tensor_copy(lmT_sb[:], lmT_ps[:, :H])
                lmT_rep = cp.tile([H, SB * BL], bf16)
                nc.vector.tensor_copy(
                    lmT_rep[:].rearrange("h (s b) -> h s b", s=SB),
                    lmT_sb[:].unsqueeze(1).broadcast_to([H, SB, BL]),
                )
                # feats rows H..2H = last_memory^T (raw, bf16)
                lmT_bf = cp.tile([H, BL], bf16)
                nc.vector.tensor_copy(lmT_bf[:], lmT_sb[:])
                nc.sync.dma_start(out=feats_local[H:H2, :], in_=lmT_bf[:])

                sc_ps = ps_acc.tile([BL, S], f32, tag="sc")
                for sb in range(NSB):
                    xt_ps = ps_t.tile([128, SB * 128], f32, tag="tps")
                    for j in range(SB):
                        s = sb * SB + j
                        nc.tensor.transpose(
                            out=xt_ps[:, j * 128:(j + 1) * 128],
                            in_=amv[:, s, :],
                            identity=ident[:],
                        )
                    xt = xtp.tile([128, SB * 128], bf16)
                    if sb % 2 == 0:
                        nc.vector.tensor_copy(xt[:], xt_ps[:])
                    else:
                        nc.scalar.copy(xt[:], xt_ps[:])
                    z_ps = ps_z.tile([128, SB * BL], f32)
                    nc.tensor.matmul(z_ps[:], lhsT=uw_sb[:], rhs=xt[:],
                                     start=True, stop=False)
                    nc.tensor.matmul(z_ps[:], lhsT=ww_sb[:], rhs=lmT_rep[:],
                                     start=False, stop=True)
                    tz = tzp.tile([128, SB * BL], bf16)
                    nc.scalar.activation(tz[:], z_ps[:], AF.Tanh)
                    for j in range(SB):
                        s = sb * SB + j
                        nc.tensor.matmul(
                            sc_ps[:, s:s + 1],
                            lhsT=tz[:, j * 128:(j + 1) * 128],
                            rhs=vw_sb[:, 0:1],
                            start=True, stop=True,
                        )

                # softmax over S (per row), normalization folded into alpha
                sc_sb = sm.tile([BL, S], f32)
                nc.vector.tensor_tensor(sc_sb[:], sc_ps[:], maskb_sb[:], op=ALU.add)
                neg_mx = sm.tile([BL, 1], f32)
                nc.vector.reduce_max(neg_mx[:], sc_sb[:], AX.X, negate=True)
                expsc = sm.tile([BL, S], f32)
                sum_sb = sm.tile([BL, 1], f32)
                nc.scalar.activation(expsc[:], sc_sb[:], AF.Exp,
                                     bias=neg_mx[:, 0:1], accum_out=sum_sb[:, 0:1])
                rsum = sm.tile([BL, 1], f32)
                nc.vector.reciprocal(rsum[:], sum_sb[:])
                alpha = sm.tile([BL, S], f32)
                nc.vector.tensor_scalar_mul(alpha[:], expsc[:], rsum[:, 0:1])

                # weighted memory: am *= alpha (broadcast over H), chunked over S
                CH = 20
                for c0 in range(0, S, CH):
                    nc.vector.tensor_tensor(
                        amv[:, c0:c0 + CH, :],
                        amv[:, c0:c0 + CH, :],
                        alpha[:, c0:c0 + CH].unsqueeze(2).broadcast_to([BL, CH, H]),
                        op=ALU.mult,
                    )
                # out_e^T [H, BL] = sum_s (alpha*am)_s^T  via PE transposes into PSUM
                oe_ps = ps_acc.tile([H, BL], f32, tag="oe")
                for s in range(S):
                    nc.tensor.matmul(oe_ps[:], lhsT=amv[:, s, :], rhs=ident[:],
                                     start=(s == 0), stop=(s == S - 1),
                                     is_transpose=True)
                oeT_sb = sm.tile([H, BL], bf16)
                nc.vector.tensor_copy(oeT_sb[:], oe_ps[:])
                nc.sync.dma_start(out=feats_local[0:H, :], in_=oeT_sb[:])

            # ---------------- AllGather feats ----------------
            if timeline:
                gvw = gath[:].rearrange("(c f) b -> c f b", f=H2)
                for c in range(NCORES):
                    nc.sync.dma_start(out=gvw[c], in_=feats_local[:])
            else:
                nc.gpsimd.collective_compute(
                    "AllGather",
                    mybir.AluOpType.bypass,
                    replica_groups=[list(range(NCORES))],
                    ins=[feats_local[:].opt()],
                    outs=[gath[:].opt()],
                )

            # ---------------- Phase 2: logits + sigmoid over local cols -------
            ew_top = ewp.tile([H, NL], bf16)
            nc.sync.dma_start(out=ew_top[:], in_=ew[0:H, :])
            ew_bot = ewp.tile([H, NL], bf16)
            nc.sync.dma_start(out=ew_bot[:], in_=ew[H:H2, :])

            with (
                tc.tile_pool(name="ps2", bufs=8, space="PSUM") as ps2,
                tc.tile_pool(name="outp", bufs=2) as outp,
                tc.tile_pool(name="gp", bufs=2) as gp,
            ):
                g_all = gp.tile([128, 2 * NCORES * 128], bf16)
                nc.sync.dma_start(
                    out=g_all[:],
                    in_=gath[:].rearrange("(t p) b -> p t b", p=128))
                # chunk grid: pairs of 512-col chunks, matmuls grouped
                # (oe,oe,lm,lm) so lhsT stays loaded across consecutive mms
                NCHW = 512
                chunks = [(n0, min(NCHW, NL - n0)) for n0 in range(0, NL, NCHW)]
                for cb in range(NCORES):
                    g_oe = g_all[:, (2 * cb) * 128:(2 * cb + 1) * 128]
                    g_lm = g_all[:, (2 * cb + 1) * 128:(2 * cb + 2) * 128]
                    mk = masks[cb]
                    out_sb = outp.tile([128, NL], bf16)
                    for p0 in range(0, len(chunks), 2):
                        pair = chunks[p0:p0 + 2]
                        pts = [ps2.tile([128, NCHW], f32, name="pt", tag="pt")
                               for _ in range(len(pair))]
                        for (n0, w), pt in zip(pair, pts):
                            nc.tensor.matmul(pt[:, :w], lhsT=g_oe,
                                             rhs=ew_top[:, n0:n0 + w],
                                             start=True, stop=False)
                        for (n0, w), pt in zip(pair, pts):
                            nc.tensor.matmul(pt[:, :w], lhsT=g_lm,
                                             rhs=ew_bot[:, n0:n0 + w],
                                             start=False, stop=True)
                        for (n0, w), pt in zip(pair, pts):
                            nc.scalar.activation(out_sb[:, n0:n0 + w],
                                                 pt[:, :w], AF.Sigmoid)
                        for (n0, w), pt in zip(pair, pts):
                            # out = (mask + 1) * sigmoid: seen -> exactly 0.0
                            nc.vector.scalar_tensor_tensor(
                                out_sb[:, n0:n0 + w],
                                mk[:, n0:n0 + w], 1.0,
                                out_sb[:, n0:n0 + w],
                                op0=ALU.add, op1=ALU.mult,
                            )
                    nc.sync.dma_start(out=out[cb * 128:(cb + 1) * 128, :],
                                      in_=out_sb[:])

    nc.compile()
    return nc


def _prepare_inputs(all_memory, last_memory, item_seq, mask, U_w, W_w, V_w, E_w):
    all_memory = np.asarray(all_memory, dtype=np.float32)
    last_memory = np.asarray(last_memory, dtype=np.float32)
    item_seq = np.asarray(item_seq)
    mask = np.asarray(mask)
    import ml_dtypes
    U_w = np.ascontiguousarray(np.asarray(U_w, dtype=np.float32).astype(ml_dtypes.bfloat16))
    W_w = np.ascontiguousarray(np.asarray(W_w, dtype=np.float32).astype(ml_dtypes.bfloat16))
    V_w = np.ascontiguousarray(np.asarray(V_w, dtype=np.float32).reshape(H, 1).astype(ml_dtypes.bfloat16))
    E_w = np.asarray(E_w, dtype=np.float32).astype(ml_dtypes.bfloat16)

    # ----- host-side mask index prep (per core, per row, per window) -----
    items = item_seq.astype(np.int64)
    valid = items > 0
    core_of = items // NL
    lcol = items - core_of * NL                          # [B,S] local column

    # max seen-items of any row within one window -> NI slots (cache key)
    NI = 8
    per_row = [[None] * B for _ in range(NCORES)]
    for c in range(NCORES):
        selc = valid & (core_of == c)
        for b in range(B):
            cols = np.unique(lcol[b][selc[b]])
            per_row[c][b] = cols
            for (w0, wlen) in WINDOWS:
                n = int(((cols >= w0) & (cols < w0 + wlen)).sum())
                NI = max(NI, n)
    NI = (NI + 1) // 2 * 2                               # even
    sidx_all = np.full((NCORES, 128, NCORES * NW * NI), -1, dtype=np.int16)
    for c in range(NCORES):
        for b in range(B):
            cb, p = b // 128, b % 128
            cols = per_row[c][b]
            for w, (w0, wlen) in enumerate(WINDOWS):
                sel = cols[(cols >= w0) & (cols < w0 + wlen)] - w0
                i0 = (cb * NW + w) * NI
                sidx_all[c, p, i0:i0 + sel.size] = sel.astype(np.int16)
    scat_w = (NI,)

    maskbias = np.where(mask, np.float32(-1e9), np.float32(0.0)).astype(np.float32)
    in_maps = []
    for c in range(NCORES):
        r0, r1 = c * BL, (c + 1) * BL
        in_maps.append({
            "am": np.ascontiguousarray(
                all_memory[r0:r1].reshape(BL, S * H)),
            "lm": np.ascontiguousarray(last_memory[r0:r1]),
            "maskb": np.ascontiguousarray(maskbias[r0:r1]),
            "uw": U_w,
            "ww": W_w,
            "vw": V_w,
            "ew": np.ascontiguousarray(E_w[:, c * NL:(c + 1) * NL]),
            "sidx": np.ascontiguousarray(sidx_all[c]),
        })
    return scat_w, in_maps


def kernel(all_memory, last_memory, item_seq, mask, U_w, W_w, V_w, V_b, E_w):
    from concourse.bass_utils import run_bass_kernel_spmd

    scat_w, in_maps = _prepare_inputs(
        all_memory, last_memory, item_seq, mask, U_w, W_w, V_w, E_w)
    if scat_w not in _BUILT:
        _BUILT[scat_w] = _build(scat_w)
    nc = _BUILT[scat_w]
    res = run_bass_kernel_spmd(nc, in_maps, core_ids=list(range(NCORES)))
    global _LAST_RESULTS
    _LAST_RESULTS = res
    return np.concatenate(
        [res.results[c]["out"].astype(np.float32) for c in range(NCORES)],
        axis=1)
